# revision 18
# baseline (speedup 1.0000x reference)
"""Multi-head attention (unfused) for one TRN2 chip (8 NeuronCores).

Sharding: 2 batches x 4 head-groups (4 heads each) = 8 cores.
Core c handles batch b = c // 4, head-group g = c % 4 (heads 4g..4g+3,
i.e. rows 256g..256g+255 of the QKV projections).

Host side pre-transposes activations to [E, S] ("xT") and weights so the
device kernel never transposes anything:
  qT = WqT.T @ xqT + bq          [256, S]   (lhsT=WqT tile, rhs=xqT tile)
  kT = WkT.T @ xkT + bk          [256, S]
  v  = xvT.T @ WvT' + bv'        [S, 260]   (WvT' interleaves a zero col per
                                             head whose bias is 1.0 -> the
                                             softmax-denominator ones column)
  per head h:
    scoresT = kT_h.T @ qT_h      [S_k, S_q] (keys on partitions; the two
                                             heads of a pair run in one
                                             2-bank psum tile via row-packed
                                             tile_position matmuls)
    expT    = exp(scoresT/8)     (one wide ScalarE op per head-pair/kt,
                                  1/sqrt(64) scale fused)
    pv      = [v_h | 1].T @ expT [65, S_q]  (row 64 = softmax denominator;
                                  runs two kt behind scores so the PE never
                                  waits on ScalarE)
    attnT_h = pv[0:64] / pv[64]  (batched reciprocal per chunk, broadcast
                                  via GpSimd partition_broadcast)
  outT_partial = WoT.T @ attnT   [E, S]  (deferred one chunk to keep the
                                          PE stream dense)
Host sums the 4 partials per batch, adds bo, transposes back.

All matmuls run with float32r operands (full-rate PE) accumulating fp32.
"""

import os
import sys

sys.path.insert(0, "/opt/trn_rl_repo")

import numpy as np

import concourse.bacc as bacc
import concourse.bass as bass
import concourse.mybir as mybir
import concourse.tile as tile
from concourse import library_config

F32 = mybir.dt.float32
F32R = mybir.dt.float32r

S = 2048          # sequence length (keys and queries)
E = 1024          # embed dim
P = 256           # projection rows per core (4 heads x 64)
D = 64            # head dim
HL = 4            # heads per core
NCORES = 8

EKT = E // 128    # 8 contraction k-tiles for projections
MT = P // 128     # 2 m-tiles for kT/qT
NSC = S // 512    # 4 s-chunks / q-chunks
NKT = S // 128    # 16 key tiles
PVW = HL * (D + 1)  # 260: v projection width incl ones columns

ROW_PACK = os.environ.get("KB_ROW_PACK", "1") == "1"


def _r(ap):
    return ap.bitcast(F32R)


def build_nc():
    nc = bacc.Bacc(trn_type="TRN2", debug=False, num_devices=NCORES,
                   enable_asserts=False)

    xq = nc.dram_tensor("xq", [E, S], F32R, kind="ExternalInput")
    xk = nc.dram_tensor("xk", [E, S], F32R, kind="ExternalInput")
    xv = nc.dram_tensor("xv", [E, S], F32R, kind="ExternalInput")
    wq = nc.dram_tensor("wq", [E, P], F32R, kind="ExternalInput")
    wk = nc.dram_tensor("wk", [E, P], F32R, kind="ExternalInput")
    wv = nc.dram_tensor("wv", [E, PVW], F32R, kind="ExternalInput")
    wo = nc.dram_tensor("wo", [P, E], F32R, kind="ExternalInput")
    bq = nc.dram_tensor("bq", [128, MT], F32, kind="ExternalInput")
    bk = nc.dram_tensor("bk", [128, MT], F32, kind="ExternalInput")
    bv = nc.dram_tensor("bv", [1, PVW], F32R, kind="ExternalInput")
    ones = nc.dram_tensor("ones", [1, 128], F32R, kind="ExternalInput")
    out = nc.dram_tensor("out", [E, S], F32, kind="ExternalOutput")

    with tile.TileContext(nc) as tc:
        with (
            tc.tile_pool(name="consts", bufs=1) as cpool,
            tc.tile_pool(name="xstage", bufs=3) as xpool,
            tc.tile_pool(name="kqv", bufs=1) as kqv_pool,
            tc.tile_pool(name="exp", bufs=5) as exp_pool,
            tc.tile_pool(name="attnsb", bufs=2) as attnsb_pool,
            tc.tile_pool(name="pvsb", bufs=2) as pvsb_pool,
            tc.tile_pool(name="small", bufs=4) as small_pool,
            tc.tile_pool(name="outstage", bufs=2) as out_pool,
            tc.tile_pool(name="ps", bufs=3, space=bass.MemorySpace.PSUM) as ps_pool,
            tc.tile_pool(name="psattn", bufs=2, space=bass.MemorySpace.PSUM) as psa_pool,
        ):
            # ---- constants ----
            wq_sb = cpool.tile([128, EKT, P], F32R, tag="wq")
            wk_sb = cpool.tile([128, EKT, P], F32R, tag="wk")
            wv_sb = cpool.tile([128, EKT, PVW], F32R, tag="wv")
            wo_sb = cpool.tile([128, MT, E], F32R, tag="wo")
            bq_sb = cpool.tile([128, MT], F32, tag="bq")
            bk_sb = cpool.tile([128, MT], F32, tag="bk")
            bv_sb = cpool.tile([1, PVW], F32R, tag="bv")
            ones_row = cpool.tile([1, 128], F32R, tag="ones")

            nc.sync.dma_start(wq_sb[:], wq.ap().rearrange("(a p) m -> p a m", p=128))
            nc.sync.dma_start(wk_sb[:], wk.ap().rearrange("(a p) m -> p a m", p=128))
            nc.sync.dma_start(wv_sb[:], wv.ap().rearrange("(a p) m -> p a m", p=128))
            nc.sync.dma_start(wo_sb[:], wo.ap().rearrange("(a p) m -> p a m", p=128))
            nc.sync.dma_start(bq_sb[:], bq.ap())
            nc.sync.dma_start(bk_sb[:], bk.ap())
            nc.sync.dma_start(bv_sb[:], bv.ap())
            nc.sync.dma_start(ones_row[:], ones.ap())
            nc.gpsimd.load_library(library_config.attn)

            kT_sb = kqv_pool.tile([128, MT, S], F32R, tag="kT")
            qT_sb = kqv_pool.tile([128, MT, S], F32R, tag="qT")
            v_sb = kqv_pool.tile([128, NKT, HL, D + 1], F32R, tag="v")

            def load_chunk(x, sc2, tag):
                t = xpool.tile([128, EKT, 512], F32R, tag="x", name="x_" + tag)
                nc.sync.dma_start(
                    t[:], x.ap()[:, sc2 * 512:(sc2 + 1) * 512]
                    .rearrange("(a p) s -> p a s", p=128))
                return t

            def proj_kq(x_t, w_sb, b_sb, dst_sb, sc2):
                # dst[:, mt, sc2*512:...] = w.T @ x + b
                ps = ps_pool.tile([128, 2, 512], F32, tag="mm",
                                  name=f"proj_{sc2}")
                for mt in range(MT):
                    for ekt in range(EKT):
                        nc.tensor.matmul(
                            ps[:, mt, :],
                            _r(w_sb[:, ekt, mt * 128:(mt + 1) * 128]),
                            _r(x_t[:, ekt, :]),
                            start=(ekt == 0), stop=(ekt == EKT - 1))
                for mt in range(MT):
                    nc.vector.tensor_scalar_add(
                        dst_sb[:, mt, sc2 * 512:(sc2 + 1) * 512],
                        ps[:, mt, :], b_sb[:, mt:mt + 1])

            def proj_v(xv_t, sc2):
                # v[st, :] = xv.T @ wv + bv, st-tiles of 128 rows.  The ones
                # columns come from zero weight columns with bias 1.0.
                for stp in range(2):
                    ps = ps_pool.tile([128, 2, 512], F32, tag="mm",
                                      name=f"vproj_{sc2}_{stp}")
                    for i in range(2):
                        sti = 2 * stp + i
                        st = sc2 * 4 + sti
                        for ekt in range(EKT):
                            nc.tensor.matmul(
                                ps[:, i, 0:PVW],
                                _r(xv_t[:, ekt, sti * 128:(sti + 1) * 128]),
                                _r(wv_sb[:, ekt, :]),
                                start=(ekt == 0), stop=False)
                        nc.tensor.matmul(
                            ps[:, i, 0:PVW], _r(ones_row[:]), _r(bv_sb[:]),
                            start=False, stop=True)
                        nc.vector.tensor_copy(
                            v_sb[:, st, :, :],
                            ps[:, i, 0:PVW].rearrange("p (h d) -> p h d", h=HL))

            def load_proj_kv(c):
                xk_t = load_chunk(xk, c, f"xk{c}")
                xv_t = load_chunk(xv, c, f"xv{c}")
                proj_kq(xk_t, wk_sb, bk_sb, kT_sb, c)
                proj_v(xv_t, c)

            def emit_outproj(sc, attn_sb):
                for mtp in range(E // 256):
                    ps_o = ps_pool.tile([128, 2, 512], F32, tag="mm",
                                        name=f"pso_{sc}_{mtp}")
                    for i in range(2):
                        mt = 2 * mtp + i
                        for kt2 in range(MT):
                            nc.tensor.matmul(
                                ps_o[:, i, :],
                                _r(wo_sb[:, kt2, mt * 128:(mt + 1) * 128]),
                                _r(attn_sb[:, kt2, :]),
                                start=(kt2 == 0), stop=(kt2 == MT - 1))
                    ot = out_pool.tile([128, 2, 512], F32, tag="ot")
                    nc.scalar.copy(ot[:], ps_o[:])
                    for i in range(2):
                        mt = 2 * mtp + i
                        nc.sync.dma_start(
                            out.ap()[mt * 128:(mt + 1) * 128,
                                     sc * 512:(sc + 1) * 512],
                            ot[:, i, :])

            class HeadPair:
                """Attention matmul pipeline for one (q-chunk, head-pair)."""

                def __init__(self, sc, hp):
                    self.sc, self.hp = sc, hp
                    self.exp_tiles = {}
                    self.attn_ps = {}
                    for i in range(2):
                        h = 2 * hp + i
                        self.attn_ps[h] = psa_pool.tile(
                            [D + 1, 512], F32, tag="pv", name=f"pv_{sc}_{h}")

                def scores(self, kt):
                    sc, hp = self.sc, self.hp
                    s_ps = ps_pool.tile([128, 2, 512], F32, tag="mm",
                                        name=f"sps_{sc}_{hp}_{kt}")
                    for i in range(2):
                        lo, hi = i * 64, (i + 1) * 64
                        nc.tensor.matmul(
                            s_ps[:, i, :],
                            _r(kT_sb[lo:hi, hp, kt * 128:(kt + 1) * 128]),
                            _r(qT_sb[lo:hi, hp, sc * 512:(sc + 1) * 512]),
                            start=True, stop=True,
                            tile_position=(lo, 0) if ROW_PACK else None)
                    exp_t = exp_pool.tile([128, 2, 512], F32R, tag="exp",
                                          name=f"exp_{sc}_{hp}_{kt}")
                    nc.scalar.activation(
                        exp_t[:], s_ps[:],
                        mybir.ActivationFunctionType.Exp, scale=0.125)
                    self.exp_tiles[kt] = exp_t

                def pv(self, kt):
                    exp_t = self.exp_tiles.pop(kt)
                    for i in range(2):
                        h = 2 * self.hp + i
                        nc.tensor.matmul(
                            self.attn_ps[h][:],
                            _r(v_sb[:, kt, h, :]),
                            _r(exp_t[:, i, :]),
                            start=(kt == 0), stop=(kt == NKT - 1))

                def step(self, kt):
                    self.scores(kt)
                    if kt > 2:
                        self.pv(kt - 3)

                def finish(self, den_sb):
                    # drain pv lag, evacuate psum, stash denominators
                    self.pv(NKT - 3)
                    self.pv(NKT - 2)
                    self.pv(NKT - 1)
                    hp = self.hp
                    pv_sb = pvsb_pool.tile([D + 1, 2, 512], F32, tag="pv_sb",
                                           name=f"pvsb_{self.sc}_{hp}")
                    for i in range(2):
                        h = 2 * hp + i
                        nc.vector.tensor_copy(pv_sb[:, i, :],
                                              self.attn_ps[h][:])
                        nc.vector.tensor_copy(
                            den_sb[32 * (2 * hp + i):32 * (2 * hp + i) + 1, :],
                            pv_sb[D:D + 1, i, :])
                    self.pv_sb = pv_sb

            def normalize(sc, pairs, den_sb, attn_sb):
                rc4 = small_pool.tile([97, 512], F32, tag="rc4",
                                      name=f"rc4_{sc}")
                nc.vector.reciprocal(rc4[:], den_sb[:])
                for hp in range(2):
                    for i in range(2):
                        h = 2 * hp + i
                        rc1 = small_pool.tile([1, 512], F32, tag="rc1",
                                              name=f"rc1_{sc}_{h}")
                        nc.vector.tensor_copy(rc1[:], rc4[32 * h:32 * h + 1, :])
                        bc = small_pool.tile([D, 512], F32, tag="bc",
                                             name=f"bc_{sc}_{h}")
                        nc.gpsimd.partition_broadcast(bc[:], rc1[:])
                        nc.vector.tensor_mul(
                            attn_sb[i * 64:(i + 1) * 64, hp, :],
                            pairs[hp].pv_sb[0:D, i, :], bc[:])

            # ---- schedule ----
            # q projection for chunk 0 first, then K/V chunk projections
            # interleaved under chunk 0's first head-pair so the input DMA
            # hides beneath attention matmuls.
            xq_t = load_chunk(xq, 0, "xq0")
            proj_kq(xq_t, wq_sb, bq_sb, qT_sb, 0)

            pending = None
            for sc in range(NSC):
                if sc + 1 < NSC:
                    xq_t = load_chunk(xq, sc + 1, f"xq{sc + 1}")
                    proj_kq(xq_t, wq_sb, bq_sb, qT_sb, sc + 1)

                attn_sb = attnsb_pool.tile([128, MT, 512], F32R, tag="attn_sb",
                                           name=f"attnsb_{sc}")
                den_sb = small_pool.tile([97, 512], F32, tag="den",
                                         name=f"den_{sc}")
                nc.vector.memset(den_sb[:], 1.0)
                pairs = []
                for hp in range(2):
                    pair = HeadPair(sc, hp)
                    pairs.append(pair)
                    if sc == 0 and hp == 0:
                        for c in range(NSC):
                            load_proj_kv(c)
                            for kt in range(4 * c, 4 * c + 4):
                                pair.step(kt)
                    else:
                        for kt in range(NKT):
                            pair.step(kt)
                    pair.finish(den_sb)
                    if hp == 0 and pending is not None:
                        # out-proj of the previous chunk rides mid-chunk so
                        # the PE filler work is split into two short blocks
                        # instead of one ScalarE-starving stretch
                        emit_outproj(*pending)
                        pending = None
                normalize(sc, pairs, den_sb, attn_sb)

                if pending is not None:
                    emit_outproj(*pending)
                pending = (sc, attn_sb)
            emit_outproj(*pending)

    nc.compile()
    return nc


_NC_CACHE = None


def _get_nc():
    global _NC_CACHE
    if _NC_CACHE is None:
        _NC_CACHE = build_nc()
    return _NC_CACHE


def make_in_maps(key, query, value, Wk, bk, Wq, bq, Wv, bv, Wo, bo):
    key = np.asarray(key, np.float32)
    query = np.asarray(query, np.float32)
    value = np.asarray(value, np.float32)
    in_maps = []
    xqT = [np.ascontiguousarray(query[b].T) for b in range(2)]
    xkT = [np.ascontiguousarray(key[b].T) for b in range(2)]
    xvT = [np.ascontiguousarray(value[b].T) for b in range(2)]
    for c in range(NCORES):
        b, g = divmod(c, 4)
        rows = slice(g * P, (g + 1) * P)
        wv_slice = np.asarray(Wv, np.float32)[rows].T  # [E, 256]
        bv_slice = np.asarray(bv, np.float32)[rows]
        wv_ext = np.zeros((E, PVW), np.float32)
        bv_ext = np.zeros((1, PVW), np.float32)
        for h in range(HL):
            wv_ext[:, h * (D + 1):h * (D + 1) + D] = wv_slice[:, h * D:(h + 1) * D]
            bv_ext[0, h * (D + 1):h * (D + 1) + D] = bv_slice[h * D:(h + 1) * D]
            bv_ext[0, h * (D + 1) + D] = 1.0
        in_maps.append({
            "xq": xqT[b],
            "xk": xkT[b],
            "xv": xvT[b],
            "wq": np.ascontiguousarray(np.asarray(Wq, np.float32)[rows].T),
            "wk": np.ascontiguousarray(np.asarray(Wk, np.float32)[rows].T),
            "wv": wv_ext,
            "wo": np.ascontiguousarray(np.asarray(Wo, np.float32)[:, rows].T),
            "bq": np.ascontiguousarray(
                np.asarray(bq, np.float32)[rows].reshape(MT, 128).T),
            "bk": np.ascontiguousarray(
                np.asarray(bk, np.float32)[rows].reshape(MT, 128).T),
            "bv": bv_ext,
            "ones": np.ones((1, 128), np.float32),
        })
    return in_maps


def assemble(results, bo):
    bo = np.asarray(bo, np.float32)
    out = np.empty((2, S, E), np.float32)
    for b in range(2):
        acc = results[4 * b]["out"].astype(np.float32).copy()
        for g in range(1, 4):
            acc += results[4 * b + g]["out"]
        out[b] = acc.T + bo[None, :]
    return out


def kernel(key, query, value, Wk, bk, Wq, bq, Wv, bv, Wo, bo):
    from concourse.bass_utils import run_bass_kernel_spmd

    nc = _get_nc()
    in_maps = make_in_maps(key, query, value, Wk, bk, Wq, bq, Wv, bv, Wo, bo)
    trace = os.environ.get("KB_TRACE", "0") == "1"
    kwargs = {}
    if trace:
        kwargs["trace"] = True
        kwargs["trace_cores"] = list(range(NCORES))
    res = run_bass_kernel_spmd(nc, in_maps, core_ids=list(range(NCORES)), **kwargs)
    if trace:
        kernel.last_results = res
    return assemble(res.results, bo)


# revision 21
# speedup vs baseline: 1.0681x; 1.0681x over previous
"""Multi-head attention (unfused) for one TRN2 chip (8 NeuronCores).

Sharding: 2 batches x 4 head-groups (4 heads each) = 8 cores.
Core c handles batch b = c // 4, head-group g = c % 4 (heads 4g..4g+3,
i.e. rows 256g..256g+255 of the QKV projections).

Host side pre-transposes activations to [E, S] ("xT") and weights so the
device kernel never transposes anything:
  qT = WqT.T @ xqT + bq          [256, S]   (lhsT=WqT tile, rhs=xqT tile)
  kT = WkT.T @ xkT + bk          [256, S]
  v  = xvT.T @ WvT' + bv'        [S, 260]   (WvT' interleaves a zero col per
                                             head whose bias is 1.0 -> the
                                             softmax-denominator ones column)
  per head h:
    scoresT = kT_h.T @ qT_h      [S_k, S_q] (keys on partitions; the two
                                             heads of a pair run in one
                                             2-bank psum tile via row-packed
                                             tile_position matmuls)
    expT    = exp(scoresT/8)     (one wide ScalarE op per head-pair/kt,
                                  1/sqrt(64) scale fused)
    pv      = [v_h | 1].T @ expT [65, S_q]  (row 64 = softmax denominator;
                                  runs two kt behind scores so the PE never
                                  waits on ScalarE)
    attnT_h = pv[0:64] / pv[64]  (batched reciprocal per chunk, broadcast
                                  via GpSimd partition_broadcast)
  outT_partial = WoT.T @ attnT   [E, S]  (deferred one chunk to keep the
                                          PE stream dense)
Host sums the 4 partials per batch, adds bo, transposes back.

All matmuls run with float32r operands (full-rate PE) accumulating fp32.
"""

import os
import sys

sys.path.insert(0, "/opt/trn_rl_repo")

import numpy as np

import concourse.bacc as bacc
import concourse.bass as bass
import concourse.mybir as mybir
import concourse.tile as tile
from concourse import library_config

F32 = mybir.dt.float32
F32R = mybir.dt.float32r

S = 2048          # sequence length (keys and queries)
E = 1024          # embed dim
P = 256           # projection rows per core (4 heads x 64)
D = 64            # head dim
HL = 4            # heads per core
NCORES = 8

EKT = E // 128    # 8 contraction k-tiles for projections
MT = P // 128     # 2 m-tiles for kT/qT
NSC = S // 512    # 4 s-chunks / q-chunks
NKT = S // 128    # 16 key tiles
PVW = HL * (D + 1)  # 260: v projection width incl ones columns

ROW_PACK = os.environ.get("KB_ROW_PACK", "1") == "1"


def _r(ap):
    return ap.bitcast(F32R)


def build_nc():
    nc = bacc.Bacc(trn_type="TRN2", debug=False, num_devices=NCORES,
                   enable_asserts=False)

    xq = nc.dram_tensor("xq", [E, S], F32R, kind="ExternalInput")
    xk = nc.dram_tensor("xk", [E, S], F32R, kind="ExternalInput")
    xv = nc.dram_tensor("xv", [E, S], F32R, kind="ExternalInput")
    wq = nc.dram_tensor("wq", [E, P], F32R, kind="ExternalInput")
    wk = nc.dram_tensor("wk", [E, P], F32R, kind="ExternalInput")
    wv = nc.dram_tensor("wv", [E, PVW], F32R, kind="ExternalInput")
    wo = nc.dram_tensor("wo", [P, E], F32R, kind="ExternalInput")
    bq = nc.dram_tensor("bq", [128, MT], F32, kind="ExternalInput")
    bk = nc.dram_tensor("bk", [128, MT], F32, kind="ExternalInput")
    bv = nc.dram_tensor("bv", [1, PVW], F32R, kind="ExternalInput")
    ones = nc.dram_tensor("ones", [1, 128], F32R, kind="ExternalInput")
    out = nc.dram_tensor("out", [E, S], F32, kind="ExternalOutput")

    with tile.TileContext(nc) as tc:
        with (
            tc.tile_pool(name="consts", bufs=1) as cpool,
            tc.tile_pool(name="xstage", bufs=3) as xpool,
            tc.tile_pool(name="kqv", bufs=1) as kqv_pool,
            tc.tile_pool(name="exp", bufs=5) as exp_pool,
            tc.tile_pool(name="attnsb", bufs=2) as attnsb_pool,
            tc.tile_pool(name="pvsb", bufs=2) as pvsb_pool,
            tc.tile_pool(name="small", bufs=4) as small_pool,
            tc.tile_pool(name="outstage", bufs=2) as out_pool,
            tc.tile_pool(name="ps", bufs=3, space=bass.MemorySpace.PSUM) as ps_pool,
            tc.tile_pool(name="psattn", bufs=2, space=bass.MemorySpace.PSUM) as psa_pool,
        ):
            # ---- constants ----
            wq_sb = cpool.tile([128, EKT, P], F32R, tag="wq")
            wk_sb = cpool.tile([128, EKT, P], F32R, tag="wk")
            wv_sb = cpool.tile([128, EKT, PVW], F32R, tag="wv")
            wo_sb = cpool.tile([128, MT, E], F32R, tag="wo")
            bq_sb = cpool.tile([128, MT], F32, tag="bq")
            bk_sb = cpool.tile([128, MT], F32, tag="bk")
            bv_sb = cpool.tile([1, PVW], F32R, tag="bv")
            ones_row = cpool.tile([1, 128], F32R, tag="ones")

            nc.sync.dma_start(wk_sb[:], wk.ap().rearrange("(a p) m -> p a m", p=128))
            nc.sync.dma_start(wv_sb[:], wv.ap().rearrange("(a p) m -> p a m", p=128))
            nc.sync.dma_start(bk_sb[:], bk.ap())
            nc.sync.dma_start(bv_sb[:], bv.ap())
            nc.sync.dma_start(ones_row[:], ones.ap())
            nc.sync.dma_start(wq_sb[:], wq.ap().rearrange("(a p) m -> p a m", p=128))
            nc.sync.dma_start(bq_sb[:], bq.ap())
            nc.sync.dma_start(wo_sb[:], wo.ap().rearrange("(a p) m -> p a m", p=128))
            nc.gpsimd.load_library(library_config.attn)

            kT_sb = kqv_pool.tile([128, MT, S], F32R, tag="kT")
            qT_sb = kqv_pool.tile([128, MT, S], F32R, tag="qT")
            v_sb = kqv_pool.tile([128, NKT, HL, D + 1], F32R, tag="v")

            def load_chunk(x, sc2, tag):
                t = xpool.tile([128, EKT, 512], F32R, tag="x", name="x_" + tag)
                nc.sync.dma_start(
                    t[:], x.ap()[:, sc2 * 512:(sc2 + 1) * 512]
                    .rearrange("(a p) s -> p a s", p=128))
                return t

            def proj_kq(x_t, w_sb, b_sb, dst_sb, sc2):
                # dst[:, mt, sc2*512:...] = w.T @ x + b
                ps = ps_pool.tile([128, 2, 512], F32, tag="mm",
                                  name=f"proj_{sc2}")
                for mt in range(MT):
                    for ekt in range(EKT):
                        nc.tensor.matmul(
                            ps[:, mt, :],
                            _r(w_sb[:, ekt, mt * 128:(mt + 1) * 128]),
                            _r(x_t[:, ekt, :]),
                            start=(ekt == 0), stop=(ekt == EKT - 1))
                for mt in range(MT):
                    nc.vector.tensor_scalar_add(
                        dst_sb[:, mt, sc2 * 512:(sc2 + 1) * 512],
                        ps[:, mt, :], b_sb[:, mt:mt + 1])

            def proj_v(xv_t, sc2):
                # v[st, :] = xv.T @ wv + bv, st-tiles of 128 rows.  The ones
                # columns come from zero weight columns with bias 1.0.
                for stp in range(2):
                    ps = ps_pool.tile([128, 2, 512], F32, tag="mm",
                                      name=f"vproj_{sc2}_{stp}")
                    for i in range(2):
                        sti = 2 * stp + i
                        st = sc2 * 4 + sti
                        for ekt in range(EKT):
                            nc.tensor.matmul(
                                ps[:, i, 0:PVW],
                                _r(xv_t[:, ekt, sti * 128:(sti + 1) * 128]),
                                _r(wv_sb[:, ekt, :]),
                                start=(ekt == 0), stop=False)
                        nc.tensor.matmul(
                            ps[:, i, 0:PVW], _r(ones_row[:]), _r(bv_sb[:]),
                            start=False, stop=True)
                        nc.vector.tensor_copy(
                            v_sb[:, st, :, :],
                            ps[:, i, 0:PVW].rearrange("p (h d) -> p h d", h=HL))

            def load_proj_kv(c):
                xk_t = load_chunk(xk, c, f"xk{c}")
                xv_t = load_chunk(xv, c, f"xv{c}")
                proj_kq(xk_t, wk_sb, bk_sb, kT_sb, c)
                proj_v(xv_t, c)

            def emit_outproj(sc, attn_sb):
                for mtp in range(E // 256):
                    ps_o = ps_pool.tile([128, 2, 512], F32, tag="mm",
                                        name=f"pso_{sc}_{mtp}")
                    for i in range(2):
                        mt = 2 * mtp + i
                        for kt2 in range(MT):
                            nc.tensor.matmul(
                                ps_o[:, i, :],
                                _r(wo_sb[:, kt2, mt * 128:(mt + 1) * 128]),
                                _r(attn_sb[:, kt2, :]),
                                start=(kt2 == 0), stop=(kt2 == MT - 1))
                    ot = out_pool.tile([128, 2, 512], F32, tag="ot")
                    nc.scalar.copy(ot[:], ps_o[:])
                    for i in range(2):
                        mt = 2 * mtp + i
                        nc.sync.dma_start(
                            out.ap()[mt * 128:(mt + 1) * 128,
                                     sc * 512:(sc + 1) * 512],
                            ot[:, i, :])

            class HeadPair:
                """Attention matmul pipeline for one (q-chunk, head-pair)."""

                def __init__(self, sc, hp):
                    self.sc, self.hp = sc, hp
                    self.exp_tiles = {}
                    self.attn_ps = {}
                    for i in range(2):
                        h = 2 * hp + i
                        self.attn_ps[h] = psa_pool.tile(
                            [D + 1, 512], F32, tag="pv", name=f"pv_{sc}_{h}")

                def scores(self, kt):
                    sc, hp = self.sc, self.hp
                    s_ps = ps_pool.tile([128, 2, 512], F32, tag="mm",
                                        name=f"sps_{sc}_{hp}_{kt}")
                    for i in range(2):
                        lo, hi = i * 64, (i + 1) * 64
                        nc.tensor.matmul(
                            s_ps[:, i, :],
                            _r(kT_sb[lo:hi, hp, kt * 128:(kt + 1) * 128]),
                            _r(qT_sb[lo:hi, hp, sc * 512:(sc + 1) * 512]),
                            start=True, stop=True,
                            tile_position=(lo, 0) if ROW_PACK else None)
                    exp_t = exp_pool.tile([128, 2, 512], F32R, tag="exp",
                                          name=f"exp_{sc}_{hp}_{kt}")
                    nc.scalar.activation(
                        exp_t[:], s_ps[:],
                        mybir.ActivationFunctionType.Exp, scale=0.125)
                    self.exp_tiles[kt] = exp_t

                def pv(self, kt):
                    exp_t = self.exp_tiles.pop(kt)
                    for i in range(2):
                        h = 2 * self.hp + i
                        nc.tensor.matmul(
                            self.attn_ps[h][:],
                            _r(v_sb[:, kt, h, :]),
                            _r(exp_t[:, i, :]),
                            start=(kt == 0), stop=(kt == NKT - 1))

                def step(self, kt):
                    self.scores(kt)
                    if kt > 2:
                        self.pv(kt - 3)

                def finish(self, den_sb):
                    # drain pv lag, evacuate psum, stash denominators
                    self.pv(NKT - 3)
                    self.pv(NKT - 2)
                    self.pv(NKT - 1)
                    hp = self.hp
                    pv_sb = pvsb_pool.tile([D + 1, 2, 512], F32, tag="pv_sb",
                                           name=f"pvsb_{self.sc}_{hp}")
                    for i in range(2):
                        h = 2 * hp + i
                        nc.vector.tensor_copy(pv_sb[:, i, :],
                                              self.attn_ps[h][:])
                        nc.vector.tensor_copy(
                            den_sb[32 * (2 * hp + i):32 * (2 * hp + i) + 1, :],
                            pv_sb[D:D + 1, i, :])
                    self.pv_sb = pv_sb

            def normalize(sc, pairs, den_sb, attn_sb):
                rc4 = small_pool.tile([97, 512], F32, tag="rc4",
                                      name=f"rc4_{sc}")
                nc.vector.reciprocal(rc4[:], den_sb[:])
                for hp in range(2):
                    for i in range(2):
                        h = 2 * hp + i
                        rc1 = small_pool.tile([1, 512], F32, tag="rc1",
                                              name=f"rc1_{sc}_{h}")
                        nc.vector.tensor_copy(rc1[:], rc4[32 * h:32 * h + 1, :])
                        bc = small_pool.tile([D, 512], F32, tag="bc",
                                             name=f"bc_{sc}_{h}")
                        nc.gpsimd.partition_broadcast(bc[:], rc1[:])
                        nc.vector.tensor_mul(
                            attn_sb[i * 64:(i + 1) * 64, hp, :],
                            pairs[hp].pv_sb[0:D, i, :], bc[:])

            # ---- schedule ----
            # q projection for chunk 0 first, then K/V chunk projections
            # interleaved under chunk 0's first head-pair so the input DMA
            # hides beneath attention matmuls.
            xq_t = load_chunk(xq, 0, "xq0")
            proj_kq(xq_t, wq_sb, bq_sb, qT_sb, 0)

            pending = None
            for sc in range(NSC):
                if sc + 1 < NSC:
                    xq_t = load_chunk(xq, sc + 1, f"xq{sc + 1}")
                    proj_kq(xq_t, wq_sb, bq_sb, qT_sb, sc + 1)

                attn_sb = attnsb_pool.tile([128, MT, 512], F32R, tag="attn_sb",
                                           name=f"attnsb_{sc}")
                den_sb = small_pool.tile([97, 512], F32, tag="den",
                                         name=f"den_{sc}")
                nc.vector.memset(den_sb[:], 1.0)
                pairs = []
                for hp in range(2):
                    pair = HeadPair(sc, hp)
                    pairs.append(pair)
                    if sc == 0 and hp == 0:
                        for c in range(NSC):
                            load_proj_kv(c)
                            for kt in range(4 * c, 4 * c + 4):
                                pair.step(kt)
                    else:
                        for kt in range(NKT):
                            pair.step(kt)
                    pair.finish(den_sb)
                normalize(sc, pairs, den_sb, attn_sb)

                if pending is not None:
                    emit_outproj(*pending)
                pending = (sc, attn_sb)
            emit_outproj(*pending)

    nc.compile()
    return nc


_NC_CACHE = None


def _get_nc():
    global _NC_CACHE
    if _NC_CACHE is None:
        _NC_CACHE = build_nc()
    return _NC_CACHE


def make_in_maps(key, query, value, Wk, bk, Wq, bq, Wv, bv, Wo, bo):
    key = np.asarray(key, np.float32)
    query = np.asarray(query, np.float32)
    value = np.asarray(value, np.float32)
    in_maps = []
    xqT = [np.ascontiguousarray(query[b].T) for b in range(2)]
    xkT = [np.ascontiguousarray(key[b].T) for b in range(2)]
    xvT = [np.ascontiguousarray(value[b].T) for b in range(2)]
    for c in range(NCORES):
        b, g = divmod(c, 4)
        rows = slice(g * P, (g + 1) * P)
        wv_slice = np.asarray(Wv, np.float32)[rows].T  # [E, 256]
        bv_slice = np.asarray(bv, np.float32)[rows]
        wv_ext = np.zeros((E, PVW), np.float32)
        bv_ext = np.zeros((1, PVW), np.float32)
        for h in range(HL):
            wv_ext[:, h * (D + 1):h * (D + 1) + D] = wv_slice[:, h * D:(h + 1) * D]
            bv_ext[0, h * (D + 1):h * (D + 1) + D] = bv_slice[h * D:(h + 1) * D]
            bv_ext[0, h * (D + 1) + D] = 1.0
        in_maps.append({
            "xq": xqT[b],
            "xk": xkT[b],
            "xv": xvT[b],
            "wq": np.ascontiguousarray(np.asarray(Wq, np.float32)[rows].T),
            "wk": np.ascontiguousarray(np.asarray(Wk, np.float32)[rows].T),
            "wv": wv_ext,
            "wo": np.ascontiguousarray(np.asarray(Wo, np.float32)[:, rows].T),
            "bq": np.ascontiguousarray(
                np.asarray(bq, np.float32)[rows].reshape(MT, 128).T),
            "bk": np.ascontiguousarray(
                np.asarray(bk, np.float32)[rows].reshape(MT, 128).T),
            "bv": bv_ext,
            "ones": np.ones((1, 128), np.float32),
        })
    return in_maps


def assemble(results, bo):
    bo = np.asarray(bo, np.float32)
    out = np.empty((2, S, E), np.float32)
    for b in range(2):
        acc = results[4 * b]["out"].astype(np.float32).copy()
        for g in range(1, 4):
            acc += results[4 * b + g]["out"]
        out[b] = acc.T + bo[None, :]
    return out


def kernel(key, query, value, Wk, bk, Wq, bq, Wv, bv, Wo, bo):
    from concourse.bass_utils import run_bass_kernel_spmd

    nc = _get_nc()
    in_maps = make_in_maps(key, query, value, Wk, bk, Wq, bq, Wv, bv, Wo, bo)
    trace = os.environ.get("KB_TRACE", "0") == "1"
    kwargs = {}
    if trace:
        kwargs["trace"] = True
        kwargs["trace_cores"] = list(range(NCORES))
    res = run_bass_kernel_spmd(nc, in_maps, core_ids=list(range(NCORES)), **kwargs)
    if trace:
        kernel.last_results = res
    return assemble(res.results, bo)
